# revision 16
# baseline (speedup 1.0000x reference)
"""Chamfer L1 distance kernel for 8 Trainium2 NeuronCores.

Problem: pred [4, 4096, 3], gt [4, 4096, 3] ->
    mean_b( mean_n min_m |pred-gt|_1  +  mean_m min_n |pred-gt|_1 )

Sharding: 8 SPMD tasks = (batch b in 0..3) x (direction in {x, y}).
Each core computes one full 4096x4096 L1 distance matrix with row-mins:
queries A (partition-tiled, 32 tiles of 128 points) vs targets B
(pre-broadcast across partitions, in the free dim). Direction is handled
purely by the per-core input map (swap pred/gt); the program is
identical on all cores. Final tiny scalar reduction happens on host.

Algorithm per query tile t (128 queries x 4096 targets):
  |v| = 2*relu(v) - v, so
  d[i,m] = sum_c |B_c[m]-A_c[i]|
         = 2*(rx+ry+rz)[i,m] - S_B[m] + S_A[i],   r_c = relu(B_c - A_c)
  DVE:  r_c = tensor_scalar(B_c, A_c[i], 0.0, sub, max)      (3 ops, 4x fp16)
  PE :  per 512-col chunk: psum = 2I @ rx + 2I @ ry + 2I @ rz
          + [ones; S_A_tile]^T @ [negS_B; ones]              (4 matmuls, fp32)
  DVE:  min tree: pairwise tensor_tensor(min) over PSUM banks -> fp16
        tree -> tensor_reduce(min) -> mins[:, t]

Toolchain constraints discovered on the way (this walrus build):
  - every compute ISA struct holds at most ONE sync wait -> single packed
    input DMA (one HW-queue semaphore), tiny "cover" ops to pre-absorb
    cross-engine ticks, and a split epilogue drain (one wait per drain).
  - AluOpType.abs_max is rejected ("is_valid_aluop") -> relu decomposition.
  - tensor_tensor_reduce (raw-ISA encoded) is rejected -> TT min tree.
  - EVENT_SEMAPHORE_RANGE_CLEAR is rejected -> skip semaphore recycling.
"""

import sys

for _p in ("/opt/trn_rl_repo",):
    if _p not in sys.path:
        sys.path.insert(0, _p)

import types

import numpy as np

import concourse.bass as bass
import concourse.tile as tile
from concourse import mybir
from concourse.vector_clock import ScopedClock

N = 4096          # points per cloud (queries == targets == 4096)
B = 4             # batch
NT = N // 128     # query tiles per core
CH = 512          # PSUM chunk (one bank of fp32)
NCH = N // CH     # chunks per tile

# packed input layout (fp16 elements per partition)
OB = 0            # Bx|By|Bz broadcast rows          [3*N]
OA = 3 * N        # acols: A[t*128+p, c] at 3t+c     [3*NT]
OW2 = OA + 3 * NT         # W2 = 2*I                 [128]
OLM = OW2 + 128           # lin moving: p0=-S_B, p1=1 [N]
OLW = OLM + N             # lin weights: p0=1, p1=S_A [N]
PK = OLW + N

F16 = mybir.dt.float16
F32 = mybir.dt.float32


def build_nc() -> bass.Bass:
    nc = bass.Bass()
    nc.clear_and_free_semaphores = lambda sems: None
    packed = nc.declare_dram_parameter("packed", [128, PK], F16, isOutput=False)
    omins = nc.declare_dram_parameter("mins", [128, NT], F32, isOutput=True)

    sub = mybir.AluOpType.subtract
    amax = mybir.AluOpType.max
    amin = mybir.AluOpType.min

    def _drain_and_barrier_split(self, tick_clock, wait_clock):
        # stock epilogue: ONE drain waiting on every pending processor,
        # which overflows the drain CTRL struct's single wait slot.
        ncc = self.nc
        drain_inst = ncc.sync.drain()
        wait_clock.add_sem_waits(
            drain_inst.ins, ScopedClock({None: tick_clock.global_clock})
        )
        si = drain_inst.ins.sync_info
        if si is not None and si.on_wait and len(si.on_wait) > 1:
            waits = list(si.on_wait)
            drain_inst.ins.sync_info = mybir.SyncInfo(
                on_wait=waits[:1], on_update=list(si.on_update or [])
            )
            for w in waits[1:]:
                d2 = ncc.sync.drain()
                d2.ins.sync_info = mybir.SyncInfo(on_wait=[w], on_update=[])
        ncc.all_engine_barrier()
        assert self.sems is not None
        popped = ncc._tile_sem_poison_stack.pop()
        assert popped is self._sem_poison
        ncc.all_engine_barrier()

    with tile.TileContext(nc) as tc:
        tc._drain_and_barrier = types.MethodType(_drain_and_barrier_split, tc)
        with (
            tc.tile_pool(name="const", bufs=1) as cpool,
            tc.tile_pool(name="work", bufs=2) as wpool,
            tc.tile_pool(name="mtree", bufs=3) as mpool,
            tc.tile_pool(name="psum", bufs=7, space="PSUM") as ppool,
            tc.tile_pool(name="psumcov", bufs=1, space="PSUM") as pcpool,
        ):
            pk_sb = cpool.tile([128, PK], F16)
            nc.sync.dma_start(pk_sb[:], packed[:])
            # per-partition scalars must be fp32 for tensor_scalar; this copy
            # also absorbs the input-DMA wait for the DVE.
            ac_sb = cpool.tile([128, 3 * NT], F32)
            nc.vector.tensor_copy(ac_sb[:], pk_sb[:, OA : OA + 3 * NT])
            mins_sb = cpool.tile([128, NT], F32)

            w2 = pk_sb[:, OW2 : OW2 + 128]

            # PE cover: tiny matmul reading only the packed tile, so the
            # first real matmul doesn't need both a DMA and a DVE wait.
            pcov = pcpool.tile([128, CH], F32, tag="pcov")
            nc.tensor.matmul(
                pcov[0:2, 0:2],
                pk_sb[0:1, 0:2],
                pk_sb[0:1, 0:2],
                start=True,
                stop=True,
            )

            for t in range(NT):
                r = []
                for c in range(3):
                    rc = wpool.tile([128, N], F16, tag=f"r{c}")
                    nc.vector.tensor_scalar(
                        rc[:],
                        pk_sb[:, c * N : (c + 1) * N],
                        ac_sb[:, 3 * t + c : 3 * t + c + 1],
                        0.0,
                        sub,
                        amax,
                    )
                    r.append(rc)
                linw = pk_sb[0:2, OLW + 128 * t : OLW + 128 * (t + 1)]

                # Fill PSUM banks in two halves of 4 so PE work on the next
                # half/tile overlaps the DVE min chain on the previous one.
                # An op may read at most one PSUM input, so the min is a
                # running chain: m_j = min(ps_j, m_{j-1}) with SBUF accum.
                m_prev = None
                for h in range(2):
                    ps = []
                    for j in range(4 * h, 4 * h + 4):
                        p = ppool.tile([128, CH], F32, tag="ps")
                        for c in range(3):
                            nc.tensor.matmul(
                                p[:],
                                w2,
                                r[c][:, j * CH : (j + 1) * CH],
                                start=(c == 0),
                                stop=False,
                            )
                        nc.tensor.matmul(
                            p[:],
                            linw,
                            pk_sb[0:2, OLM + j * CH : OLM + (j + 1) * CH],
                            start=False,
                            stop=True,
                        )
                        ps.append(p)
                    # DVE cover: absorb this half's PE tick so each chain op
                    # below carries only its single merged DVE wait.
                    cov = mpool.tile([1, 2], F32, tag="cov")
                    nc.vector.tensor_copy(cov[:], ps[3][0:1, 0:2])
                    for k, p in enumerate(ps):
                        m = mpool.tile([128, CH], F16, tag=f"m{(4 * h + k) % 2}")
                        if m_prev is None:
                            nc.vector.tensor_copy(m[:], p[:])
                        else:
                            nc.vector.tensor_tensor(m[:], p[:], m_prev[:], amin)
                        m_prev = m
                nc.vector.tensor_reduce(
                    mins_sb[:, t : t + 1],
                    m_prev[:],
                    axis=mybir.AxisListType.X,
                    op=amin,
                )

            nc.sync.dma_start(omins[:], mins_sb[:])
    return nc


def make_in_maps(pred: np.ndarray, gt: np.ndarray) -> list[dict[str, np.ndarray]]:
    """Core c handles batch c//2; even cores: queries=pred, targets=gt
    (cham_x); odd cores: queries=gt, targets=pred (cham_y)."""
    in_maps = []
    for c in range(8):
        b = c // 2
        A, T = (pred[b], gt[b]) if c % 2 == 0 else (gt[b], pred[b])
        A = np.asarray(A, dtype=np.float32)
        T = np.asarray(T, dtype=np.float32)
        packed = np.zeros((128, PK), dtype=np.float16)
        packed[:, OB : OB + 3 * N] = (
            T.T.astype(np.float16).reshape(1, 3 * N)
        )
        # acols: A[t*128+p, c] at [p, t*3+c]
        packed[:, OA : OA + 3 * NT] = (
            A.reshape(NT, 128, 3).transpose(1, 0, 2).reshape(128, 3 * NT)
        ).astype(np.float16)
        packed[np.arange(128), OW2 + np.arange(128)] = 2.0
        packed[0, OLM : OLM + N] = (-T.sum(1)).astype(np.float16)
        packed[1, OLM : OLM + N] = 1.0
        packed[0, OLW : OLW + N] = 1.0
        packed[1, OLW : OLW + N] = A.sum(1).astype(np.float16)
        in_maps.append({"packed": packed})
    return in_maps


_NC_CACHE: list = []


def run_spmd(in_maps, trace: bool = False):
    from concourse.bass_utils import run_bass_kernel_spmd

    if not _NC_CACHE:
        _NC_CACHE.append(build_nc())
    nc = _NC_CACHE[0]
    return run_bass_kernel_spmd(nc, in_maps, core_ids=list(range(8)), trace=trace)


def kernel(pred: np.ndarray, gt: np.ndarray) -> np.ndarray:
    pred = np.asarray(pred, dtype=np.float32)
    gt = np.asarray(gt, dtype=np.float32)
    res = run_spmd(make_in_maps(pred, gt))
    total = 0.0
    for c in range(8):
        total += float(res.results[c]["mins"].sum(dtype=np.float64))
    return np.asarray(total / (N * B), dtype=np.float32)
